# revision 51
# baseline (speedup 1.0000x reference)
"""Distributed Trainium2 (Bass/Tile) kernel for nn_Attention_2D.

Pipeline (per batch element): 3x3 conv + BatchNorm (batch stats!) for
Q (from x), K, V (from y) -> linear projections -> multi-head attention
(scale = C**-0.5) -> output projection.

Sharding: data-parallel over batch B=8 across the 8 NeuronCores (one
image per core). The only cross-core dependency is the BatchNorm
mean/var over the whole batch -> a tiny [128,12] AllReduce.

Device layout notes:
  - images are stored channel-major [C, L] (C on partitions, 2 chunks of
    128), so BN is a per-partition affine and conv = 9 shifted matmuls
    with weight tiles [ci, co].
  - conv inputs live in a zero-padded [c, 34, 34] buffer so the 9 shifts
    are strided access patterns of one buffer.
  - attention is computed in the transposed orientation S^T[t, l] with t
    on partitions; 4 heads run concurrently in the PE array via
    row-tiling (K=32 each).  exp() runs on ScalarE straight out of PSUM
    with the 1/16 scale folded in, writing bf16 probabilities P^T.
  - attn@V uses col-tiling (M=32 per head, 4 heads concurrent) to
    produce the output directly transposed [c, l]; an all-ones [128,32]
    stationary operand produces the softmax denominators pre-broadcast
    across each head's 32 partitions, so normalization is one
    reciprocal + one multiply.
  - matmuls use float32r (full PE speed at N>=256, fp32 storage; DMA
    into an f32r tile performs the rounding the BIR verifier requires).
"""

import numpy as np

B, L, C = 8, 1024, 256
H = 8
D = 32  # head dim
IMG = 32  # h = w = 32
PAD = 34  # padded image side
EPS = 1e-5
ATT_SCALE = float(C) ** -0.5  # 1/16

_CACHE = {}
DEBUG = False
VARIANT = "full"  # "full" | "noattn" | "convonly" (phase timing builds)


def _build_nc(repeat=1):
    import concourse.bacc as bacc
    import concourse.tile as tile
    from concourse import mybir

    f32 = mybir.dt.float32
    f32r = mybir.dt.float32r
    bf16 = mybir.dt.bfloat16
    f8 = mybir.dt.float8e4
    DR = mybir.MatmulPerfMode.DoubleRow
    AF = mybir.ActivationFunctionType
    ALU = mybir.AluOpType

    nc = bacc.Bacc(None, target_bir_lowering=False)
    nc.num_devices = 8

    # ---- DRAM parameters (host-prepped layouts) ----
    # Q/K-path weights+activations are fp8e4 (host scales weights by
    # WSCALE=16 so they clear e4m3's subnormal zone; BN absorbs the conv-
    # weight scale exactly and the projection scales fold into the exp
    # scale).  The V path stays f32r/bf16: the near-uniform softmax means the
    # attention output is ~30x smaller than V, so fp8 error there does NOT
    # average down relative to the output.  Score errors OTOH are divided by
    # 16 and barely move the softmax, so the Q/K path tolerates fp8.
    # x[b].T zero-padded to 34x34 (host bakes the conv padding)
    xt = nc.declare_dram_parameter("xt", [C, PAD * PAD], f8, isOutput=False)
    yt8 = nc.declare_dram_parameter("yt8", [C, PAD * PAD], f8, isOutput=False)
    ytr = nc.declare_dram_parameter("ytr", [C, PAD * PAD], f32r, isOutput=False)
    # fp8 conv weights: [9(kpos), 2(co), 128(p=ci_in), 2(ci chunk), 128(f)]
    # w8[kp,co,p,ci,f] = conv_w[co*128+f, ci*128+p, ky, kx] * 16
    wcq = nc.declare_dram_parameter("wcq", [9, 2, 128, 2, 128], f8, isOutput=False)
    wck = nc.declare_dram_parameter("wck", [9, 2, 128, 2, 128], f8, isOutput=False)
    # f32r conv weights: [9(kpos), 2(ci), 2(co), 128, 128] with w[kp,ci,co,p,f]
    # = conv_w[co*128+f, ci*128+p, ky, kx]
    wcv = nc.declare_dram_parameter("wcv", [9, 2, 2, 128, 128], f32r, isOutput=False)
    # projection weights W.T tiled: [2(ci), 128, 256(co)]; q/k fp8 * 16
    pq = nc.declare_dram_parameter("pq", [2, 128, C], f8, isOutput=False)
    pk = nc.declare_dram_parameter("pk", [2, 128, C], f8, isOutput=False)
    pv = nc.declare_dram_parameter("pv", [2, 128, C], f32r, isOutput=False)
    # Wo^T row-permuted + zero-padded to the attnA/B layout:
    # po[tX, g, pb+d, f] = Wo[f, g*128 + j(tX,pb)*32 + d] for pb in {0,64},
    # j = tX*2 + (pb//64); rows 32-63 and 96-127 are zero.  bf16 (as is the
    # attn rhs) to stay off the f32r-rounding verifier path.
    po = nc.declare_dram_parameter("po", [2, 2, 128, C], bf16, isOutput=False)
    # gamma/beta pack [128, 12]: cols 0-5 gamma, 6-11 beta, col order
    # (q c0, q c1, k c0, k c1, v c0, v c1)
    gb = nc.declare_dram_parameter("gb", [128, 12], f32, isOutput=False)
    bo = nc.declare_dram_parameter("bo", [128, 2], f32, isOutput=False)
    out = nc.declare_dram_parameter("out", [C, L], f32, isOutput=True)
    dbg = {}
    if DEBUG:
        for name, shape, dt_ in (
            ("dkraw", [128, 2 * L], f32), ("dkbn", [128, 2 * L], f32),
            ("dst", [128, 12], f32), ("dgst", [128, 12], f32),
            ("dscale", [128, 6], f32), ("dshift", [128, 6], f32),
            ("dqT", [128, 2 * L], f32), ("dkT", [128, 2 * L], f32),
            ("dav", [128, 512], f32), ("ddsb", [128, 2, 512], mybir.dt.bfloat16),
            ("drepb", [128, 512], f32), ("drsb", [128, 512], f32),
            ("daoT", [128, 2 * L], mybir.dt.bfloat16),
        ):
            dbg[name] = nc.declare_dram_parameter(name, shape, dt_, isOutput=True)

    with tile.TileContext(nc) as tc:
        with tc.tile_pool(name="singles", bufs=1) as singles, \
             tc.tile_pool(name="stats", bufs=1) as statsp, \
             tc.tile_pool(name="bnst", bufs=4) as bnstp, \
             tc.tile_pool(name="rep", bufs=3) as repp, \
             tc.tile_pool(name="pt", bufs=3) as ptp, \
             tc.tile_pool(name="ps", bufs=3, space="PSUM") as psp, \
             tc.tile_pool(name="score_ps", bufs=2, space="PSUM") as scorep, \
             tc.tile_pool(name="dram", bufs=1, space="DRAM") as dramp:

            for _rep in range(repeat):
                # ---------- constants / small tiles ----------
                # all-ones stationary for the den-row partition broadcasts
                # (built via rounding copies so the f32r matmul verifier is
                # satisfied), plus a 1/32-valued one for the second hop of
                # the rows-64..95 broadcast (PE tile positions must be
                # row-banded or col-banded, never diagonal, so (96,64) is
                # illegal and the odd heads broadcast in two hops).
                onesb = singles.tile([128, 128], bf16)
                nc.vector.memset(onesb[:], 1.0)
                sc128b = singles.tile([128, 64], bf16)
                nc.vector.memset(sc128b[:], 1.0 / 128.0)
                epst = singles.tile([128, 1], f32)
                nc.vector.memset(epst[:], EPS)
                gbt = singles.tile([128, 12], f32)
                nc.sync.dma_start(out=gbt[:], in_=gb[:])
                bot = singles.tile([128, 2], f32)
                nc.sync.dma_start(out=bot[:], in_=bo[:])

                # ---------- padded images + weights ----------
                # Two HWDGE rings run in parallel and each ring is FIFO, so
                # emit in consumption order: the q path (conv_q runs first) on
                # the sync ring, the k/v path on the scalar ring.
                pad_x = singles.tile([128, 2, PAD, PAD], f8)
                pad_y8 = singles.tile([128, 2, PAD, PAD], f8)
                pad_yv = singles.tile([128, 2, PAD, PAD], f32r)
                wq_sb = singles.tile([128, 9, 2, 2, 128], f8)
                wk_sb = singles.tile([128, 9, 2, 2, 128], f8)
                wv_sb = singles.tile([128, 36 * 128], f32r)
                pq_sb = singles.tile([128, 2, C], f8)
                pk_sb = singles.tile([128, 2, C], f8)
                pv_sb = singles.tile([128, 2 * C], f32r)
                po_sb = singles.tile([128, 2, 2, C], bf16)

                ytr8 = yt8.rearrange("(c p) m -> p c m", p=128)
                ytrr = ytr.rearrange("(c p) m -> p c m", p=128)
                xtr = xt.rearrange("(c p) m -> p c m", p=128)
                wckr = wck.rearrange("(a k) b p c f -> p a k b c f", a=3)
                wcqr = wcq.rearrange("(a k) b p c f -> p a k b c f", a=3)
                wv4 = wv_sb[:].rearrange("p (a t f) -> p a t f", a=3, f=128)
                wcvr = wcv.rearrange("(a g) b c p f -> p a (g b c) f", a=3)
                # sync ring: pads for conv_k first; scalar ring: wk chunks —
                # both arrive in parallel so conv_k starts early.
                for ci in range(2):
                    nc.sync.dma_start(out=pad_y8[:, ci], in_=ytr8[:, ci])
                for a in range(3):
                    nc.scalar.dma_start(out=wk_sb[:, 3 * a: 3 * a + 3], in_=wckr[:, a])
                for ci in range(2):
                    nc.scalar.dma_start(out=pad_x[:, ci], in_=xtr[:, ci])
                for a in range(3):
                    nc.sync.dma_start(out=wq_sb[:, 3 * a: 3 * a + 3], in_=wcqr[:, a])
                for ci in range(2):
                    nc.sync.dma_start(out=pad_yv[:, ci], in_=ytrr[:, ci])
                for a in range(3):
                    nc.scalar.dma_start(out=wv4[:, a], in_=wcvr[:, a])
                nc.scalar.dma_start(out=pq_sb[:], in_=pq.rearrange("t p f -> p t f"))
                nc.sync.dma_start(out=pk_sb[:], in_=pk.rearrange("t p f -> p t f"))
                nc.sync.dma_start(
                    out=pv_sb[:].rearrange("p (t f) -> p t f", f=C),
                    in_=pv.rearrange("t p f -> p t f"))
                nc.sync.dma_start(out=po_sb[:],
                                  in_=po.rearrange("t g p f -> p t g f"))

                # ---------- conv: raw = conv(img) in [c, L] layout ----------
                # raw tiles [128, 2048], col = chunk*1024 + l
                kraw = singles.tile([128, 2 * L], f32)
                vraw = singles.tile([128, 2 * L], f32)
                qraw = singles.tile([128, 2 * L], f32)
                st = statsp.tile([128, 12], f32)  # local (mean, m2) pairs

                def bn_local_stats(raw, stat_base):
                    # local BN statistics per chunk -> st cols (mean, m2)
                    for ch in range(2):
                        k = stat_base + ch
                        st6 = bnstp.tile([128, 2, 6], f32, tag="st6")
                        nc.vector.bn_stats(st6[:, 0, :], raw[:, ch * L: ch * L + 512])
                        nc.vector.bn_stats(st6[:, 1, :], raw[:, ch * L + 512: ch * L + 1024])
                        nc.vector.bn_aggr(st[:, 2 * k: 2 * k + 2], st6[:])
                        # m2 = mean^2 + var  (in place on the var column)
                        nc.vector.scalar_tensor_tensor(
                            out=st[:, 2 * k + 1: 2 * k + 2],
                            in0=st[:, 2 * k: 2 * k + 1],
                            scalar=st[:, 2 * k: 2 * k + 1],
                            in1=st[:, 2 * k + 1: 2 * k + 2],
                            op0=ALU.mult, op1=ALU.add,
                        )

                def conv8(pad_t, w_sb, raw, stat_base):
                    # fp8 DoubleRow conv: ktile dim pairs the two ci chunks
                    for co in range(2):
                        for half in range(2):
                            ps = psp.tile([128, 512], f32, tag="ps")
                            for kp in range(9):
                                ky, kx = kp // 3, kp % 3
                                rhs = pad_t[:, :, ky + half * 16: ky + half * 16 + 16,
                                            kx: kx + 32]
                                nc.tensor.matmul(ps[:], w_sb[:, kp, co], rhs,
                                                 start=(kp == 0), stop=(kp == 8),
                                                 perf_mode=DR)
                            nc.vector.tensor_copy(
                                out=raw[:, co * L + half * 512: co * L + (half + 1) * 512],
                                in_=ps[:])
                    bn_local_stats(raw, stat_base)

                def convr(pad_t, w_sb, raw, stat_base):
                    for co in range(2):
                        for half in range(2):
                            ps = psp.tile([128, 512], f32, tag="ps")
                            idx = 0
                            for kp in range(9):
                                ky, kx = kp // 3, kp % 3
                                for ci in range(2):
                                    blk = (kp * 2 + ci) * 2 + co
                                    lhsT = w_sb[:, blk * 128:(blk + 1) * 128]
                                    rhs = pad_t[:, ci, ky + half * 16: ky + half * 16 + 16,
                                                kx: kx + 32]
                                    nc.tensor.matmul(ps[:], lhsT, rhs,
                                                     start=(idx == 0), stop=(idx == 17))
                                    idx += 1
                            nc.vector.tensor_copy(
                                out=raw[:, co * L + half * 512: co * L + (half + 1) * 512].bitcast(f32r),
                                in_=ps[:])
                    bn_local_stats(raw, stat_base)

                conv8(pad_y8, wk_sb, kraw, 2)
                conv8(pad_x, wq_sb, qraw, 0)

                # ---------- AllReduce #1: q+k stats (overlaps conv_v) --------
                # q and k are all the exp stream needs; the v path (CC2 +
                # v-projection) hides under the attention exps since only the
                # attn@V matmuls consume it and the PE can catch up.
                cc_in1 = dramp.tile([128, 8], f32)
                cc_out1 = dramp.tile([128, 8], f32)
                nc.sync.dma_start(out=cc_in1[:], in_=st[:, 0:8])
                nc.gpsimd.collective_compute(
                    "AllReduce", ALU.add,
                    replica_groups=[list(range(8))],
                    ins=[cc_in1[:].opt()], outs=[cc_out1[:].opt()],
                )
                gstats = statsp.tile([128, 12], f32)
                nc.sync.dma_start(out=gstats[:, 0:8], in_=cc_out1[:])

                convr(pad_yv, wv_sb, vraw, 4)

                # ---------- AllReduce #2: v stats ----------
                cc_in2 = dramp.tile([128, 4], f32)
                cc_out2 = dramp.tile([128, 4], f32)
                nc.sync.dma_start(out=cc_in2[:], in_=st[:, 8:12])
                nc.gpsimd.collective_compute(
                    "AllReduce", ALU.add,
                    replica_groups=[list(range(8))],
                    ins=[cc_in2[:].opt()], outs=[cc_out2[:].opt()],
                )
                nc.sync.dma_start(out=gstats[:, 8:12], in_=cc_out2[:])

                if DEBUG:
                    nc.sync.dma_start(out=dbg["dkraw"][:], in_=kraw[:])
                    nc.sync.dma_start(out=dbg["dst"][:], in_=st[:])

                # ---------- global scale/shift ----------
                var_t = statsp.tile([128, 6], f32)
                scale_t = statsp.tile([128, 6], f32)
                shift_t = statsp.tile([128, 6], f32)

                def bn_post(k0, nk):
                    seg = gstats[:, 2 * k0: 2 * (k0 + nk)]
                    nc.vector.tensor_scalar_mul(seg, seg, 1.0 / 8.0)
                    g2 = seg.rearrange("p (k two) -> p k two", two=2)
                    gmean = g2[:, :, 0]
                    gm2 = g2[:, :, 1]
                    vt = var_t[:, k0: k0 + nk]
                    nc.vector.tensor_mul(vt, gmean, gmean)
                    nc.vector.tensor_sub(vt, gm2, vt)
                    # rstd = exp(-0.5 * ln(var + eps)); ln+exp share one ACT
                    # table set so the big attention exps need no reload
                    nc.scalar.activation(vt, vt, AF.Ln, bias=epst[:, 0:1], scale=1.0)
                    nc.scalar.activation(vt, vt, AF.Exp, scale=-0.5)
                    sc = scale_t[:, k0: k0 + nk]
                    sh = shift_t[:, k0: k0 + nk]
                    nc.vector.tensor_mul(sc, vt, gbt[:, k0: k0 + nk])
                    nc.vector.tensor_mul(sh, gmean, sc)
                    nc.vector.tensor_sub(sh, gbt[:, 6 + k0: 6 + k0 + nk], sh)

                def bn_apply8(raw, raw8, base):
                    # BN'd activation emitted as fp8 for the DR projection
                    for ch in range(2):
                        k = base + ch
                        nc.vector.tensor_scalar(
                            out=raw8[:, ch, :],
                            in0=raw[:, ch * L:(ch + 1) * L],
                            scalar1=scale_t[:, k: k + 1],
                            scalar2=shift_t[:, k: k + 1],
                            op0=ALU.mult, op1=ALU.add,
                        )

                def bn_apply(raw, base):
                    for ch in range(2):
                        k = base + ch
                        nc.vector.tensor_scalar(
                            out=raw[:, ch * L:(ch + 1) * L].bitcast(f32r),
                            in0=raw[:, ch * L:(ch + 1) * L],
                            scalar1=scale_t[:, k: k + 1],
                            scalar2=shift_t[:, k: k + 1],
                            op0=ALU.mult, op1=ALU.add,
                        )

                qraw8 = singles.tile([128, 2, L], f8)
                kraw8 = singles.tile([128, 2, L], f8)
                bn_post(0, 4)   # q, k (CC1 results; overlaps conv_v / CC2)
                bn_apply8(qraw, qraw8, 0)
                bn_apply8(kraw, kraw8, 2)

                if DEBUG:
                    nc.sync.dma_start(out=dbg["dgst"][:], in_=gstats[:])
                    nc.sync.dma_start(out=dbg["dscale"][:], in_=scale_t[:])
                    nc.sync.dma_start(out=dbg["dshift"][:], in_=shift_t[:])
                    nc.sync.dma_start(out=dbg["dkbn"][:], in_=kraw[:])

                # ---------- q/k projections -> transposed [c, L] ----------
                qT = singles.tile([128, 2 * L], f32)
                kT = singles.tile([128, 2 * L], f32)

                def proj_T(src8, wsb, dst, co):
                    # fp8 DoubleRow projection: ktile pairs the two ci chunks
                    for lh in range(2):
                        ps = psp.tile([128, 512], f32, tag="ps")
                        nc.tensor.matmul(ps[:], wsb[:, :, co * 128:(co + 1) * 128],
                                         src8[:, :, lh * 512:(lh + 1) * 512],
                                         start=True, stop=True, perf_mode=DR)
                        nc.scalar.copy(
                            dst[:, co * L + lh * 512: co * L + (lh + 1) * 512].bitcast(f32r),
                            ps[:])

                # co-chunk-interleaved so attention group g=0 (which needs the
                # co=0 halves of BOTH kT and qT) is fed first
                for co in range(2):
                    proj_T(kraw8, pk_sb, kT, co)   # overlaps conv_v / CC2
                    proj_T(qraw8, pq_sb, qT, co)
                bn_post(4, 2)               # v (CC2 results)
                bn_apply(vraw, 4)

                # ---------- v projection -> [t, g, head, 33] bf16 ----------
                # col 32 of each head slot is 1.0: attn@V with this stationary
                # (M=33) also produces the softmax denominator in out row 32,
                # so no separate ones-matmul stream is needed.
                v1_sb = singles.tile([128, 8, 2, 4, 33], bf16)
                nc.vector.memset(v1_sb[:, :, :, :, 32:33], 1.0)
                for lt in range(8):
                    ps = psp.tile([128, C], f32, tag="ps")
                    for ci in range(2):
                        lhsT = vraw[:, ci * L + lt * 128: ci * L + (lt + 1) * 128].bitcast(f32r)
                        rhs = pv_sb[:, ci * C:(ci + 1) * C]
                        nc.tensor.matmul(ps[:], lhsT, rhs, start=(ci == 0), stop=(ci == 1))
                    nc.vector.tensor_copy(
                        out=v1_sb[:, lt, :, :, 0:32],
                        in_=ps[:].rearrange("p (g j d) -> p g j d", g=2, j=4))

                if DEBUG:
                    nc.sync.dma_start(out=dbg["dqT"][:], in_=qT[:])
                    nc.sync.dma_start(out=dbg["dkT"][:], in_=kT[:])

                if VARIANT == "convonly":
                    nc.sync.dma_start(
                        out=out.rearrange("(c p) l -> p c l", p=128),
                        in_=kraw[:].rearrange("p (c l) -> p c l", l=L))
                    continue
                if VARIANT == "noattn":
                    nc.sync.dma_start(
                        out=out.rearrange("(c p) l -> p c l", p=128),
                        in_=qT[:].rearrange("p (c l) -> p c l", l=L))
                    continue

                # ---------- attention ----------
                # Score tiles hold a PAIR of heads, one PSUM bank per head.
                # exp runs on ScalarE straight out of PSUM (no staging copy),
                # emitting bf16 probabilities that feed attn@V directly.  The
                # 1/256 compensates the x16 fp8 scaling of Wq and Wk.
                # attn@V uses the ones-augmented V (M=33): row 32 of each
                # head's 64-col band is the denominator, which a 1-row ones
                # matmul broadcasts back over the head's 32 partitions.
                # Heads land at partition rows {0-31, 64-95} of attnA (heads
                # 0,1 of each group) / attnB (heads 2,3); the out projection
                # compensates with a row-permuted, zero-padded Wo.
                attnA = singles.tile([128, 2 * L], bf16)  # col = chunk*1024 + l
                attnB = singles.tile([128, 2 * L], bf16)
                nc.vector.memset(attnA[:], 0.0)
                nc.vector.memset(attnB[:], 0.0)
                for g in range(2):
                    for lh in range(2):
                        av33a = psp.tile([128, 512], f32, tag="ps")
                        av33b = psp.tile([128, 512], f32, tag="ps")
                        av33 = [av33a, av33b]
                        nc.vector.memset(av33a[:], 0.0)
                        nc.vector.memset(av33b[:], 0.0)
                        for tc_i in range(8):
                            for jp in range(2):  # head pairs (0,1), (2,3)
                                score = scorep.tile([128, 2, 512], f32, tag="score")
                                for jj in range(2):
                                    j = 2 * jp + jj
                                    lhsT = kT[32 * j: 32 * j + 32,
                                              g * L + tc_i * 128: g * L + (tc_i + 1) * 128].bitcast(f32r)
                                    rhs = qT[32 * j: 32 * j + 32,
                                             g * L + lh * 512: g * L + (lh + 1) * 512].bitcast(f32r)
                                    nc.tensor.matmul(score[:, jj, :],
                                                     lhsT, rhs, start=True, stop=True,
                                                     tile_position=(32 * j, 0))
                                pt = ptp.tile([128, 2, 512], bf16, tag="pt")
                                nc.scalar.activation(pt[:], score[:],
                                                     AF.Exp, scale=ATT_SCALE / 256.0)
                                for jj in range(2):
                                    j = 2 * jp + jj
                                    pos = 64 * (j % 2)
                                    nc.tensor.matmul(
                                        av33[j // 2][pos: pos + 33, :],
                                        v1_sb[:, tc_i, g, j, :], pt[:, jj, :],
                                        start=False, stop=False,
                                        tile_position=(0, pos),
                                        skip_group_check=True)
                        # den rows (32, 96) -> SBUF so the broadcast matmuls
                        # can consume them as moving data
                        dsb = repp.tile([128, 2, 512], bf16, tag="dsb")
                        for ti in range(2):
                            for pb in (32, 96):
                                nc.vector.tensor_copy(
                                    out=dsb[pb: pb + 1, ti, :],
                                    in_=av33[ti][pb: pb + 1, :])
                        gcol = slice(g * L + lh * 512, g * L + (lh + 1) * 512)
                        for ti, attnX in ((0, attnA), (1, attnB)):
                            repb = psp.tile([128, 512], f32, tag="ps")
                            # PE tiling is strictly 1-D (row- OR col-banded),
                            # so the odd head's den (row 96 -> rows 64-95)
                            # broadcasts in hops of valid shapes: (96,0) M=128
                            # floods all rows, a full copy stages to SBUF,
                            # then a (0,64) col-banded matmul with a
                            # 1/128-summing stationary rebuilds rows 64-95;
                            # (32,0) finally overwrites rows 0-31 with the
                            # even head's den.
                            nc.tensor.matmul(
                                repb[:, :], onesb[96:97, :],
                                dsb[96:97, ti, :],
                                start=True, stop=True,
                                tile_position=(96, 0), skip_group_check=True)
                            s32 = repp.tile([128, 512], bf16, tag="rsb")
                            nc.vector.tensor_copy(out=s32[:], in_=repb[:])
                            nc.tensor.matmul(
                                repb[64:128, :], sc128b[:, :], s32[:],
                                start=True, stop=True,
                                tile_position=(0, 64), skip_group_check=True)
                            nc.tensor.matmul(
                                repb[0:32, :], onesb[32:33, 0:32],
                                dsb[32:33, ti, :],
                                start=True, stop=True,
                                tile_position=(32, 0), skip_group_check=True)
                            rsb = repp.tile([128, 512], f32, tag="rsb")
                            # custom-DVE ops misbehave at nonzero base
                            # partition; run the reciprocal over the full
                            # tile (rows 32-63/96-127 hold finite den-scale
                            # junk, never read)
                            nc.vector.reciprocal_approx_fast(
                                out=rsb[:], in_=repb[:])
                            if DEBUG and g == 0 and lh == 0 and ti == 0:
                                dsc = repp.tile([128, 512], f32, tag="rsb")
                                nc.vector.tensor_copy(out=dsc[:], in_=av33[0][:])
                                nc.sync.dma_start(out=dbg["dav"][:], in_=dsc[:])
                                nc.sync.dma_start(out=dbg["ddsb"][:], in_=dsb[:])
                                dsc2 = repp.tile([128, 512], f32, tag="rsb")
                                nc.vector.tensor_copy(out=dsc2[:], in_=repb[:])
                                nc.sync.dma_start(out=dbg["drepb"][:], in_=dsc2[:])
                            for hb in (0, 64):
                                nc.vector.tensor_mul(
                                    attnX[hb: hb + 32, gcol],
                                    av33[ti][hb: hb + 32, :],
                                    rsb[hb: hb + 32, :])

                if DEBUG:
                    nc.sync.dma_start(out=dbg["daoT"][:], in_=attnA[:])
                    nc.sync.dma_start(out=dbg["drsb"][:], in_=rsb[:])

                # ---------- output projection (transposed) + bias ----------
                # po_sb rows are permuted/zero-padded to match the attnA/B
                # row layout, so rows 32-63/96-127 (zeroed at memset) multiply
                # by zero weights and the contraction can stay K=128.
                out_sb = singles.tile([128, 2 * L], f32)
                for lh in range(2):
                    for co in range(2):
                        ps = psp.tile([128, 512], f32, tag="ps")
                        idx = 0
                        for tX, attnX in ((0, attnA), (1, attnB)):
                            for gci in range(2):
                                lhsT = po_sb[:, tX, gci, co * 128:(co + 1) * 128]
                                rhs = attnX[:, gci * L + lh * 512:
                                            gci * L + (lh + 1) * 512]
                                nc.tensor.matmul(ps[:], lhsT, rhs,
                                                 start=(idx == 0), stop=(idx == 3))
                                idx += 1
                        nc.scalar.activation(
                            out_sb[:, co * L + lh * 512: co * L + (lh + 1) * 512],
                            ps[:], AF.Identity, bias=bot[:, co: co + 1], scale=1.0)

                outr = out.rearrange("(c p) l -> p c l", p=128)
                osr = out_sb[:].rearrange("p (c l) -> p c l", l=L)
                for lh in range(2):
                    nc.sync.dma_start(out=outr[:, :, lh * 512:(lh + 1) * 512],
                                      in_=osr[:, :, lh * 512:(lh + 1) * 512])

    nc.compile()
    return nc


WSCALE = 16.0


def _f8(a):
    import ml_dtypes
    return np.ascontiguousarray(a).astype(ml_dtypes.float8_e4m3)


def _prep_weights(conv_q_w, conv_k_w, conv_v_w, Wq, Wk, Wv, Wo,
                  bn_q_g, bn_q_b, bn_k_g, bn_k_b, bn_v_g, bn_v_b, bo):
    def conv_tiles(w):
        # [co, ci, ky, kx] -> [9, 2(ci), 2(co), 128, 128]
        t = np.ascontiguousarray(np.transpose(np.asarray(w, np.float32), (2, 3, 1, 0)))
        t = t.reshape(3, 3, 2, 128, 2, 128).transpose(0, 1, 2, 4, 3, 5)
        return np.ascontiguousarray(t.reshape(9, 2, 2, 128, 128))

    def conv_tiles8(w):
        # [co, ci, ky, kx] -> [9, 2(co), 128(ci_in), 2(ci ch), 128(co_in)] fp8
        t = np.transpose(np.asarray(w, np.float32), (2, 3, 1, 0))  # ky kx ci co
        t = t.reshape(3, 3, 2, 128, 2, 128)        # ky kx cic cip coc cof
        t = t.transpose(0, 1, 4, 3, 2, 5)          # ky kx coc cip cic cof
        return _f8(t.reshape(9, 2, 128, 2, 128) * WSCALE)

    def proj_tiles(w):
        return np.ascontiguousarray(
            np.asarray(w, np.float32).T.reshape(2, 128, C))

    def proj_tiles8(w):
        return _f8(np.asarray(w, np.float32).T.reshape(2, 128, C) * WSCALE)

    def po_tiles(w):
        # row-permuted, zero-padded Wo^T matching the attnA/B row layout
        import ml_dtypes
        wt = np.asarray(w, np.float32).T  # [cin, fo]
        p = np.zeros((2, 2, 128, C), np.float32)
        for tX in range(2):
            for g in range(2):
                for pb in (0, 64):
                    j = tX * 2 + pb // 64
                    p[tX, g, pb: pb + 32, :] = wt[g * 128 + j * 32:
                                                  g * 128 + (j + 1) * 32, :]
        return np.ascontiguousarray(p).astype(ml_dtypes.bfloat16)

    gbp = np.zeros((128, 12), np.float32)
    for i, (g, b) in enumerate(((bn_q_g, bn_q_b), (bn_k_g, bn_k_b), (bn_v_g, bn_v_b))):
        g = np.asarray(g, np.float32).reshape(2, 128)
        b = np.asarray(b, np.float32).reshape(2, 128)
        for ch in range(2):
            gbp[:, 2 * i + ch] = g[ch]
            gbp[:, 6 + 2 * i + ch] = b[ch]
    bop = np.ascontiguousarray(np.asarray(bo, np.float32).reshape(2, 128).T)
    return {
        "wcq": conv_tiles8(conv_q_w), "wck": conv_tiles8(conv_k_w),
        "wcv": conv_tiles(conv_v_w),
        "pq": proj_tiles8(Wq), "pk": proj_tiles8(Wk), "pv": proj_tiles(Wv),
        "po": po_tiles(Wo),
        "gb": gbp, "bo": bop,
    }


def _get_nc(repeat=1):
    key = ("nc", repeat, VARIANT, DEBUG)
    if key not in _CACHE:
        _CACHE[key] = _build_nc(repeat)
    return _CACHE[key]


def run_spmd(in_maps, repeat=1, **kw):
    from concourse.bass_utils import run_bass_kernel_spmd
    return run_bass_kernel_spmd(_get_nc(repeat), in_maps, list(range(8)), **kw)


def _get_executor(repeat=1):
    """Build the sharded jitted callable once (mirrors
    bass2jax.run_bass_via_pjrt's multi-core path) so repeated calls skip
    retracing/compilation."""
    key = ("exec", repeat, VARIANT)
    if key in _CACHE:
        return _CACHE[key]
    import jax
    import numpy as _np
    from jax.sharding import Mesh, PartitionSpec
    from jax.experimental.shard_map import shard_map
    from concourse import bass2jax, mybir

    nc = _get_nc(repeat)
    bass2jax.install_neuronx_cc_hook()
    partition_name = nc.partition_id_tensor.name if nc.partition_id_tensor else None

    in_names, out_names, out_avals, zero_outs = [], [], [], []
    for alloc in nc.m.functions[0].allocations:
        if not isinstance(alloc, mybir.MemoryLocationSet):
            continue
        name = alloc.memorylocations[0].name
        if alloc.kind == "ExternalInput":
            if name != partition_name:
                in_names.append(name)
        elif alloc.kind == "ExternalOutput":
            dt_np = mybir.dt.np(alloc.dtype)
            shape = tuple(alloc.tensor_shape)
            out_avals.append(jax.core.ShapedArray(shape, dt_np))
            out_names.append(name)
            zero_outs.append(_np.zeros(shape, dt_np))

    n_params = len(in_names)
    n_outs = len(out_names)
    all_in_names = list(in_names) + list(out_names)
    if partition_name is not None:
        all_in_names.append(partition_name)
    donate = tuple(range(n_params, n_params + n_outs))

    def _body(*args):
        operands = list(args)
        if partition_name is not None:
            operands.append(bass2jax.partition_id_tensor())
        outs = bass2jax._bass_exec_p.bind(
            *operands,
            out_avals=tuple(out_avals),
            in_names=tuple(all_in_names),
            out_names=tuple(out_names),
            lowering_input_output_aliases=(),
            sim_require_finite=True,
            sim_require_nnan=True,
            nc=nc,
        )
        return tuple(outs)

    devices = jax.devices()[:B]
    mesh = Mesh(np.asarray(devices), ("core",))
    in_specs = (PartitionSpec("core"),) * (n_params + n_outs)
    out_specs = (PartitionSpec("core"),) * n_outs
    sharded = jax.jit(
        shard_map(_body, mesh=mesh, in_specs=in_specs, out_specs=out_specs,
                  check_rep=False),
        donate_argnums=donate, keep_unused=True,
    )
    _CACHE[("mesh", repeat, VARIANT)] = mesh
    _CACHE[("jit", repeat, VARIANT)] = sharded

    def run(in_maps):
        concat_in = [
            np.concatenate([np.asarray(in_maps[c][k]) for c in range(B)], axis=0)
            for k in in_names
        ]
        concat_zeros = [np.zeros((B * z.shape[0], *z.shape[1:]), z.dtype)
                        for z in zero_outs]
        out_arrs = sharded(*concat_in, *concat_zeros)
        return out_arrs, out_names, out_avals

    _CACHE[key] = run
    return run


def run_fast(in_maps, repeat=1):
    """Execute via the cached jitted callable; returns per-core dict list."""
    run = _get_executor(repeat)
    out_arrs, out_names, out_avals = run(in_maps)
    return [
        {name: np.asarray(out_arrs[i]).reshape(B, *out_avals[i].shape)[c]
         for i, name in enumerate(out_names)}
        for c in range(B)
    ]


def bench_wall(in_maps, repeat, n_iter):
    """Dispatch n_iter executions of the repeat-R NEFF with device-resident
    inputs and pre-staged donated zero buffers; return total wall seconds.
    Host/RPC overhead is identical across R, so (wall(R2)-wall(R1)) isolates
    device time."""
    import time as _time
    import jax
    from jax.sharding import NamedSharding, PartitionSpec

    _get_executor(repeat)  # ensure built
    nc = _get_nc(repeat)
    from concourse import mybir
    partition_name = nc.partition_id_tensor.name if nc.partition_id_tensor else None
    in_names, out_shapes = [], []
    for alloc in nc.m.functions[0].allocations:
        if not isinstance(alloc, mybir.MemoryLocationSet):
            continue
        name = alloc.memorylocations[0].name
        if alloc.kind == "ExternalInput" and name != partition_name:
            in_names.append(name)
        elif alloc.kind == "ExternalOutput":
            out_shapes.append((tuple(alloc.tensor_shape), mybir.dt.np(alloc.dtype)))

    key = ("bench_in", repeat, VARIANT)
    if key not in _CACHE:
        run = _CACHE[("exec", repeat, VARIANT)]
        # reach into the executor's jitted fn? rebuild inputs here instead
        mesh = _CACHE[("mesh", repeat, VARIANT)]
        sh = NamedSharding(mesh, PartitionSpec("core"))
        dev_in = [
            jax.device_put(
                np.concatenate([np.asarray(in_maps[c][k]) for c in range(B)], 0), sh)
            for k in in_names
        ]
        _CACHE[key] = (dev_in, sh)
    dev_in, sh = _CACHE[key]

    sharded = _CACHE[("jit", repeat, VARIANT)]
    # pre-stage donated zero sets
    zero_sets = []
    for _ in range(n_iter):
        zs = [jax.device_put(np.zeros((B * s[0], *s[1:]), dt), sh)
              for (s, dt) in out_shapes]
        zero_sets.append(zs)
    for zs in zero_sets:
        for z in zs:
            z.block_until_ready()

    outs = []
    t0 = _time.perf_counter()
    for it in range(n_iter):
        outs.append(sharded(*dev_in, *zero_sets[it]))
    for o in outs[-1]:
        o.block_until_ready()
    t1 = _time.perf_counter()
    return t1 - t0


def make_in_maps(x, y, h, w, conv_q_w, bn_q_g, bn_q_b,
                 conv_k_w, bn_k_g, bn_k_b, conv_v_w, bn_v_g, bn_v_b,
                 Wq, Wk, Wv, Wo, bo):
    assert int(h) == IMG and int(w) == IMG
    x = np.asarray(x, np.float32)
    y = np.asarray(y, np.float32)
    wmap = _prep_weights(conv_q_w, conv_k_w, conv_v_w, Wq, Wk, Wv, Wo,
                         bn_q_g, bn_q_b, bn_k_g, bn_k_b, bn_v_g, bn_v_b, bo)
    def pad_t(a):
        # [B, L, C] -> [B, C, 34*34] with zero border baked in
        at = np.transpose(a, (0, 2, 1)).reshape(B, C, IMG, IMG)
        ap = np.zeros((B, C, PAD, PAD), np.float32)
        ap[:, :, 1:33, 1:33] = at
        return ap.reshape(B, C, PAD * PAD)

    xT = _f8(pad_t(x))
    yT = pad_t(y)
    yT8 = _f8(yT)
    return [dict(wmap, xt=xT[b], yt8=yT8[b], ytr=yT[b]) for b in range(B)]


def kernel(**inputs):
    in_maps = make_in_maps(**inputs)
    res = run_fast(in_maps)
    outs = [res[b]["out"] for b in range(B)]  # each [C, L]
    return np.ascontiguousarray(
        np.stack(outs, axis=0).transpose(0, 2, 1)).astype(np.float32)



# revision 52
# speedup vs baseline: 1.1894x; 1.1894x over previous
"""Distributed Trainium2 (Bass/Tile) kernel for nn_Attention_2D.

Pipeline (per batch element): 3x3 conv + BatchNorm (batch stats!) for
Q (from x), K, V (from y) -> linear projections -> multi-head attention
(scale = C**-0.5) -> output projection.

Sharding: data-parallel over batch B=8 across the 8 NeuronCores (one
image per core). The only cross-core dependency is the BatchNorm
mean/var over the whole batch -> a tiny [128,12] AllReduce.

Device layout notes:
  - images are stored channel-major [C, L] (C on partitions, 2 chunks of
    128), so BN is a per-partition affine and conv = 9 shifted matmuls
    with weight tiles [ci, co].
  - conv inputs live in a zero-padded [c, 34, 34] buffer so the 9 shifts
    are strided access patterns of one buffer.
  - attention is computed in the transposed orientation S^T[t, l] with t
    on partitions; 4 heads run concurrently in the PE array via
    row-tiling (K=32 each).  exp() runs on ScalarE straight out of PSUM
    with the 1/16 scale folded in, writing bf16 probabilities P^T.
  - attn@V uses col-tiling (M=32 per head, 4 heads concurrent) to
    produce the output directly transposed [c, l]; an all-ones [128,32]
    stationary operand produces the softmax denominators pre-broadcast
    across each head's 32 partitions, so normalization is one
    reciprocal + one multiply.
  - matmuls use float32r (full PE speed at N>=256, fp32 storage; DMA
    into an f32r tile performs the rounding the BIR verifier requires).
"""

import numpy as np

B, L, C = 8, 1024, 256
H = 8
D = 32  # head dim
IMG = 32  # h = w = 32
PAD = 34  # padded image side
EPS = 1e-5
ATT_SCALE = float(C) ** -0.5  # 1/16

_CACHE = {}
DEBUG = False
VARIANT = "full"  # "full" | "noattn" | "convonly" (phase timing builds)


def _build_nc(repeat=1):
    import concourse.bacc as bacc
    import concourse.tile as tile
    from concourse import mybir

    f32 = mybir.dt.float32
    f32r = mybir.dt.float32r
    bf16 = mybir.dt.bfloat16
    f8 = mybir.dt.float8e4
    DR = mybir.MatmulPerfMode.DoubleRow
    AF = mybir.ActivationFunctionType
    ALU = mybir.AluOpType

    nc = bacc.Bacc(None, target_bir_lowering=False)
    nc.num_devices = 8

    # ---- DRAM parameters (host-prepped layouts) ----
    # Q/K-path weights+activations are fp8e4 (host scales weights by
    # WSCALE=16 so they clear e4m3's subnormal zone; BN absorbs the conv-
    # weight scale exactly and the projection scales fold into the exp
    # scale).  The V path stays f32r/bf16: the near-uniform softmax means the
    # attention output is ~30x smaller than V, so fp8 error there does NOT
    # average down relative to the output.  Score errors OTOH are divided by
    # 16 and barely move the softmax, so the Q/K path tolerates fp8.
    # x[b].T zero-padded to 34x34 (host bakes the conv padding)
    xt = nc.declare_dram_parameter("xt", [C, PAD * PAD], f8, isOutput=False)
    yt8 = nc.declare_dram_parameter("yt8", [C, PAD * PAD], f8, isOutput=False)
    ytr = nc.declare_dram_parameter("ytr", [C, PAD * PAD], f32r, isOutput=False)
    # fp8 conv weights: [9(kpos), 2(co), 128(p=ci_in), 2(ci chunk), 128(f)]
    # w8[kp,co,p,ci,f] = conv_w[co*128+f, ci*128+p, ky, kx] * 16
    wcq = nc.declare_dram_parameter("wcq", [9, 2, 128, 2, 128], f8, isOutput=False)
    wck = nc.declare_dram_parameter("wck", [9, 2, 128, 2, 128], f8, isOutput=False)
    # f32r conv weights: [9(kpos), 2(ci), 2(co), 128, 128] with w[kp,ci,co,p,f]
    # = conv_w[co*128+f, ci*128+p, ky, kx]
    wcv = nc.declare_dram_parameter("wcv", [9, 2, 2, 128, 128], f32r, isOutput=False)
    # projection weights W.T tiled: [2(ci), 128, 256(co)]; q/k fp8 * 16
    pq = nc.declare_dram_parameter("pq", [2, 128, C], f8, isOutput=False)
    pk = nc.declare_dram_parameter("pk", [2, 128, C], f8, isOutput=False)
    pv = nc.declare_dram_parameter("pv", [2, 128, C], f32r, isOutput=False)
    # Wo^T row-permuted + zero-padded to the attnA/B layout:
    # po[tX, g, pb+d, f] = Wo[f, g*128 + j(tX,pb)*32 + d] for pb in {0,64},
    # j = tX*2 + (pb//64); rows 32-63 and 96-127 are zero.  bf16 (as is the
    # attn rhs) to stay off the f32r-rounding verifier path.
    po = nc.declare_dram_parameter("po", [2, 2, 128, C], bf16, isOutput=False)
    # gamma/beta pack [128, 12]: cols 0-5 gamma, 6-11 beta, col order
    # (q c0, q c1, k c0, k c1, v c0, v c1)
    gb = nc.declare_dram_parameter("gb", [128, 12], f32, isOutput=False)
    bo = nc.declare_dram_parameter("bo", [128, 2], f32, isOutput=False)
    out = nc.declare_dram_parameter("out", [C, L], f32, isOutput=True)
    dbg = {}
    if DEBUG:
        for name, shape, dt_ in (
            ("dkraw", [128, 2 * L], f32), ("dkbn", [128, 2 * L], f32),
            ("dst", [128, 12], f32), ("dgst", [128, 12], f32),
            ("dscale", [128, 6], f32), ("dshift", [128, 6], f32),
            ("dqT", [128, 2 * L], f32), ("dkT", [128, 2 * L], f32),
            ("dav", [128, 512], f32), ("ddsb", [128, 2, 512], mybir.dt.bfloat16),
            ("drepb", [128, 512], f32), ("drsb", [128, 512], f32),
            ("daoT", [128, 2 * L], mybir.dt.bfloat16),
        ):
            dbg[name] = nc.declare_dram_parameter(name, shape, dt_, isOutput=True)

    with tile.TileContext(nc) as tc:
        with tc.tile_pool(name="singles", bufs=1) as singles, \
             tc.tile_pool(name="stats", bufs=1) as statsp, \
             tc.tile_pool(name="bnst", bufs=4) as bnstp, \
             tc.tile_pool(name="rep", bufs=3) as repp, \
             tc.tile_pool(name="pt", bufs=35) as ptp, \
             tc.tile_pool(name="ps", bufs=3, space="PSUM") as psp, \
             tc.tile_pool(name="score_ps", bufs=2, space="PSUM") as scorep, \
             tc.tile_pool(name="dram", bufs=1, space="DRAM") as dramp:

            for _rep in range(repeat):
                # ---------- constants / small tiles ----------
                # all-ones stationary for the den-row partition broadcasts
                # (built via rounding copies so the f32r matmul verifier is
                # satisfied), plus a 1/32-valued one for the second hop of
                # the rows-64..95 broadcast (PE tile positions must be
                # row-banded or col-banded, never diagonal, so (96,64) is
                # illegal and the odd heads broadcast in two hops).
                onesb = singles.tile([128, 128], bf16)
                nc.vector.memset(onesb[:], 1.0)
                sc128b = singles.tile([128, 64], bf16)
                nc.vector.memset(sc128b[:], 1.0 / 128.0)
                epst = singles.tile([128, 1], f32)
                nc.vector.memset(epst[:], EPS)
                gbt = singles.tile([128, 12], f32)
                nc.sync.dma_start(out=gbt[:], in_=gb[:])
                bot = singles.tile([128, 2], f32)
                nc.sync.dma_start(out=bot[:], in_=bo[:])

                # ---------- padded images + weights ----------
                # Two HWDGE rings run in parallel and each ring is FIFO, so
                # emit in consumption order: the q path (conv_q runs first) on
                # the sync ring, the k/v path on the scalar ring.
                pad_x = singles.tile([128, 2, PAD, PAD], f8)
                pad_y8 = singles.tile([128, 2, PAD, PAD], f8)
                pad_yv = singles.tile([128, 2, PAD, PAD], f32r)
                wq_sb = singles.tile([128, 9, 2, 2, 128], f8)
                wk_sb = singles.tile([128, 9, 2, 2, 128], f8)
                wv_sb = singles.tile([128, 36 * 128], f32r)
                pq_sb = singles.tile([128, 2, C], f8)
                pk_sb = singles.tile([128, 2, C], f8)
                pv_sb = singles.tile([128, 2 * C], f32r)
                po_sb = singles.tile([128, 2, 2, C], bf16)

                ytr8 = yt8.rearrange("(c p) m -> p c m", p=128)
                ytrr = ytr.rearrange("(c p) m -> p c m", p=128)
                xtr = xt.rearrange("(c p) m -> p c m", p=128)
                wckr = wck.rearrange("(a k) b p c f -> p a k b c f", a=3)
                wcqr = wcq.rearrange("(a k) b p c f -> p a k b c f", a=3)
                wv4 = wv_sb[:].rearrange("p (a t f) -> p a t f", a=3, f=128)
                wcvr = wcv.rearrange("(a g) b c p f -> p a (g b c) f", a=3)
                # sync ring: pads for conv_k first; scalar ring: wk chunks —
                # both arrive in parallel so conv_k starts early.
                for ci in range(2):
                    nc.sync.dma_start(out=pad_y8[:, ci], in_=ytr8[:, ci])
                for a in range(3):
                    nc.scalar.dma_start(out=wk_sb[:, 3 * a: 3 * a + 3], in_=wckr[:, a])
                for ci in range(2):
                    nc.scalar.dma_start(out=pad_x[:, ci], in_=xtr[:, ci])
                for a in range(3):
                    nc.sync.dma_start(out=wq_sb[:, 3 * a: 3 * a + 3], in_=wcqr[:, a])
                for ci in range(2):
                    nc.sync.dma_start(out=pad_yv[:, ci], in_=ytrr[:, ci])
                for a in range(3):
                    nc.scalar.dma_start(out=wv4[:, a], in_=wcvr[:, a])
                nc.scalar.dma_start(out=pq_sb[:], in_=pq.rearrange("t p f -> p t f"))
                nc.sync.dma_start(out=pk_sb[:], in_=pk.rearrange("t p f -> p t f"))
                nc.sync.dma_start(
                    out=pv_sb[:].rearrange("p (t f) -> p t f", f=C),
                    in_=pv.rearrange("t p f -> p t f"))
                nc.sync.dma_start(out=po_sb[:],
                                  in_=po.rearrange("t g p f -> p t g f"))

                # ---------- conv: raw = conv(img) in [c, L] layout ----------
                # raw tiles [128, 2048], col = chunk*1024 + l
                kraw = singles.tile([128, 2 * L], f32)
                vraw = singles.tile([128, 2 * L], f32)
                qraw = singles.tile([128, 2 * L], f32)
                st = statsp.tile([128, 12], f32)  # local (mean, m2) pairs

                def bn_local_stats(raw, stat_base):
                    # local BN statistics per chunk -> st cols (mean, m2)
                    for ch in range(2):
                        k = stat_base + ch
                        st6 = bnstp.tile([128, 2, 6], f32, tag="st6")
                        nc.vector.bn_stats(st6[:, 0, :], raw[:, ch * L: ch * L + 512])
                        nc.vector.bn_stats(st6[:, 1, :], raw[:, ch * L + 512: ch * L + 1024])
                        nc.vector.bn_aggr(st[:, 2 * k: 2 * k + 2], st6[:])
                        # m2 = mean^2 + var  (in place on the var column)
                        nc.vector.scalar_tensor_tensor(
                            out=st[:, 2 * k + 1: 2 * k + 2],
                            in0=st[:, 2 * k: 2 * k + 1],
                            scalar=st[:, 2 * k: 2 * k + 1],
                            in1=st[:, 2 * k + 1: 2 * k + 2],
                            op0=ALU.mult, op1=ALU.add,
                        )

                def conv8(pad_t, w_sb, raw, stat_base):
                    # fp8 DoubleRow conv: ktile dim pairs the two ci chunks
                    for co in range(2):
                        for half in range(2):
                            ps = psp.tile([128, 512], f32, tag="ps")
                            for kp in range(9):
                                ky, kx = kp // 3, kp % 3
                                rhs = pad_t[:, :, ky + half * 16: ky + half * 16 + 16,
                                            kx: kx + 32]
                                nc.tensor.matmul(ps[:], w_sb[:, kp, co], rhs,
                                                 start=(kp == 0), stop=(kp == 8),
                                                 perf_mode=DR)
                            nc.vector.tensor_copy(
                                out=raw[:, co * L + half * 512: co * L + (half + 1) * 512],
                                in_=ps[:])
                    bn_local_stats(raw, stat_base)

                def convr_co(pad_t, w_sb, raw, co):
                    if True:
                        for half in range(2):
                            ps = psp.tile([128, 512], f32, tag="ps")
                            idx = 0
                            for kp in range(9):
                                ky, kx = kp // 3, kp % 3
                                for ci in range(2):
                                    blk = (kp * 2 + ci) * 2 + co
                                    lhsT = w_sb[:, blk * 128:(blk + 1) * 128]
                                    rhs = pad_t[:, ci, ky + half * 16: ky + half * 16 + 16,
                                                kx: kx + 32]
                                    nc.tensor.matmul(ps[:], lhsT, rhs,
                                                     start=(idx == 0), stop=(idx == 17))
                                    idx += 1
                            nc.vector.tensor_copy(
                                out=raw[:, co * L + half * 512: co * L + (half + 1) * 512].bitcast(f32r),
                                in_=ps[:])

                conv8(pad_y8, wk_sb, kraw, 2)
                conv8(pad_x, wq_sb, qraw, 0)
                prefills = {}

                # ---------- AllReduce #1: q+k stats (overlaps conv_v) --------
                # q and k are all the exp stream needs; the v path (CC2 +
                # v-projection) hides under the attention exps since only the
                # attn@V matmuls consume it and the PE can catch up.
                cc_in1 = dramp.tile([128, 8], f32)
                cc_out1 = dramp.tile([128, 8], f32)
                nc.sync.dma_start(out=cc_in1[:], in_=st[:, 0:8])
                nc.gpsimd.collective_compute(
                    "AllReduce", ALU.add,
                    replica_groups=[list(range(8))],
                    ins=[cc_in1[:].opt()], outs=[cc_out1[:].opt()],
                )
                gstats = statsp.tile([128, 12], f32)
                nc.sync.dma_start(out=gstats[:, 0:8], in_=cc_out1[:])

                convr_co(pad_yv, wv_sb, vraw, 0)  # fills the CC1 gap

                if DEBUG:
                    nc.sync.dma_start(out=dbg["dkraw"][:], in_=kraw[:])
                    nc.sync.dma_start(out=dbg["dst"][:], in_=st[:])

                # ---------- global scale/shift ----------
                var_t = statsp.tile([128, 6], f32)
                scale_t = statsp.tile([128, 6], f32)
                shift_t = statsp.tile([128, 6], f32)

                def bn_post(k0, nk):
                    seg = gstats[:, 2 * k0: 2 * (k0 + nk)]
                    nc.vector.tensor_scalar_mul(seg, seg, 1.0 / 8.0)
                    g2 = seg.rearrange("p (k two) -> p k two", two=2)
                    gmean = g2[:, :, 0]
                    gm2 = g2[:, :, 1]
                    vt = var_t[:, k0: k0 + nk]
                    nc.vector.tensor_mul(vt, gmean, gmean)
                    nc.vector.tensor_sub(vt, gm2, vt)
                    # rstd = exp(-0.5 * ln(var + eps)); ln+exp share one ACT
                    # table set so the big attention exps need no reload
                    nc.scalar.activation(vt, vt, AF.Ln, bias=epst[:, 0:1], scale=1.0)
                    nc.scalar.activation(vt, vt, AF.Exp, scale=-0.5)
                    sc = scale_t[:, k0: k0 + nk]
                    sh = shift_t[:, k0: k0 + nk]
                    nc.vector.tensor_mul(sc, vt, gbt[:, k0: k0 + nk])
                    nc.vector.tensor_mul(sh, gmean, sc)
                    nc.vector.tensor_sub(sh, gbt[:, 6 + k0: 6 + k0 + nk], sh)

                def bn_apply8(raw, raw8, base):
                    # BN'd activation emitted as fp8 for the DR projection
                    for ch in range(2):
                        k = base + ch
                        nc.vector.tensor_scalar(
                            out=raw8[:, ch, :],
                            in0=raw[:, ch * L:(ch + 1) * L],
                            scalar1=scale_t[:, k: k + 1],
                            scalar2=shift_t[:, k: k + 1],
                            op0=ALU.mult, op1=ALU.add,
                        )

                def bn_apply(raw, base):
                    for ch in range(2):
                        k = base + ch
                        nc.vector.tensor_scalar(
                            out=raw[:, ch * L:(ch + 1) * L].bitcast(f32r),
                            in0=raw[:, ch * L:(ch + 1) * L],
                            scalar1=scale_t[:, k: k + 1],
                            scalar2=shift_t[:, k: k + 1],
                            op0=ALU.mult, op1=ALU.add,
                        )

                qraw8 = singles.tile([128, 2, L], f8)
                kraw8 = singles.tile([128, 2, L], f8)
                bn_post(0, 4)   # q, k (CC1 results; overlaps conv_v / CC2)
                bn_apply8(qraw, qraw8, 0)
                bn_apply8(kraw, kraw8, 2)

                if DEBUG:
                    nc.sync.dma_start(out=dbg["dgst"][:], in_=gstats[:])
                    nc.sync.dma_start(out=dbg["dscale"][:], in_=scale_t[:])
                    nc.sync.dma_start(out=dbg["dshift"][:], in_=shift_t[:])
                    nc.sync.dma_start(out=dbg["dkbn"][:], in_=kraw[:])

                # ---------- q/k projections -> fp8 DoubleRow layout ----------
                # q8dr[32j+p, g, i, l] = q[c = g*128 + 32j + 16i + p, l] for
                # p<16; rows 32j+16..32j+32 are d-hi staging scratch.
                q8dr = singles.tile([128, 2, 2, L], f8)
                k8dr = singles.tile([128, 2, 2, L], f8)

                def proj_T(src8, wsb, dst, co):
                    # fp8 DoubleRow projection: ktile pairs the two ci chunks.
                    # The full PSUM lands in ktile slot 0 as fp8 (head j's
                    # d-lo sits on its 16-row band; the d-hi rows double as
                    # staging), then per-head DMAs shift d-hi into slot 1.
                    for lh in range(2):
                        ps = psp.tile([128, 512], f32, tag="ps")
                        nc.tensor.matmul(ps[:], wsb[:, :, co * 128:(co + 1) * 128],
                                         src8[:, :, lh * 512:(lh + 1) * 512],
                                         start=True, stop=True, perf_mode=DR)
                        lhc = slice(lh * 512, (lh + 1) * 512)
                        nc.scalar.copy(dst[:, co, 0, lhc], ps[:])
                        for j in range(4):
                            eng = nc.sync if j % 2 else nc.scalar
                            eng.dma_start(
                                out=dst[32 * j: 32 * j + 16, co, 1, lhc],
                                in_=dst[32 * j + 16: 32 * j + 32, co, 0, lhc])

                # co-chunk-interleaved so attention group g=0 (which needs the
                # co=0 halves of BOTH kT and qT) is fed first
                for co in range(2):
                    proj_T(kraw8, pk_sb, k8dr, co)
                    proj_T(qraw8, pq_sb, q8dr, co)

                def score_exp(g, lh, tc_i, jp):
                    score = scorep.tile([128, 2, 512], f32, tag="score")
                    for jj in range(2):
                        j = 2 * jp + jj
                        lhsT = k8dr[32 * j: 32 * j + 16, g, :,
                                    tc_i * 128:(tc_i + 1) * 128]
                        rhs = q8dr[32 * j: 32 * j + 16, g, :,
                                   lh * 512:(lh + 1) * 512]
                        nc.tensor.matmul(score[:, jj, :],
                                         lhsT, rhs, start=True, stop=True,
                                         perf_mode=DR,
                                         tile_position=(32 * j, 0))
                    pt = ptp.tile([128, 2, 512], bf16, tag="pt")
                    nc.scalar.activation(pt[:], score[:],
                                         AF.Exp, scale=ATT_SCALE / 256.0)
                    return pt

                # prefill the (g=0, lh) score+exp streams so ScalarE is busy
                # while the PE finishes conv_v and the v projection
                for lh_pre in range(2):
                    prefills[(0, lh_pre)] = [score_exp(0, lh_pre, t, jp)
                                             for t in range(8) for jp in range(2)]

                convr_co(pad_yv, wv_sb, vraw, 1)
                bn_local_stats(vraw, 4)

                # ---------- AllReduce #2: v stats ----------
                cc_in2 = dramp.tile([128, 4], f32)
                cc_out2 = dramp.tile([128, 4], f32)
                nc.sync.dma_start(out=cc_in2[:], in_=st[:, 8:12])
                nc.gpsimd.collective_compute(
                    "AllReduce", ALU.add,
                    replica_groups=[list(range(8))],
                    ins=[cc_in2[:].opt()], outs=[cc_out2[:].opt()],
                )
                nc.sync.dma_start(out=gstats[:, 8:12], in_=cc_out2[:])
                bn_post(4, 2)               # v (CC2 results)
                bn_apply(vraw, 4)

                # ---------- v projection -> [t, g, head, 33] bf16 ----------
                # col 32 of each head slot is 1.0: attn@V with this stationary
                # (M=33) also produces the softmax denominator in out row 32,
                # so no separate ones-matmul stream is needed.
                v1_sb = singles.tile([128, 8, 2, 4, 33], bf16)
                nc.vector.memset(v1_sb[:, :, :, :, 32:33], 1.0)
                for lt in range(8):
                    ps = psp.tile([128, C], f32, tag="ps")
                    for ci in range(2):
                        lhsT = vraw[:, ci * L + lt * 128: ci * L + (lt + 1) * 128].bitcast(f32r)
                        rhs = pv_sb[:, ci * C:(ci + 1) * C]
                        nc.tensor.matmul(ps[:], lhsT, rhs, start=(ci == 0), stop=(ci == 1))
                    nc.vector.tensor_copy(
                        out=v1_sb[:, lt, :, :, 0:32],
                        in_=ps[:].rearrange("p (g j d) -> p g j d", g=2, j=4))


                if VARIANT == "convonly":
                    nc.sync.dma_start(
                        out=out.rearrange("(c p) l -> p c l", p=128),
                        in_=kraw[:].rearrange("p (c l) -> p c l", l=L))
                    continue
                if VARIANT == "noattn":
                    nc.sync.dma_start(
                        out=out.rearrange("(c p) l -> p c l", p=128),
                        in_=qraw[:].rearrange("p (c l) -> p c l", l=L))
                    continue

                # ---------- attention ----------
                # Score tiles hold a PAIR of heads, one PSUM bank per head.
                # exp runs on ScalarE straight out of PSUM (no staging copy),
                # emitting bf16 probabilities that feed attn@V directly.  The
                # 1/256 compensates the x16 fp8 scaling of Wq and Wk.
                # attn@V uses the ones-augmented V (M=33): row 32 of each
                # head's 64-col band is the denominator, which a 1-row ones
                # matmul broadcasts back over the head's 32 partitions.
                # Heads land at partition rows {0-31, 64-95} of attnA (heads
                # 0,1 of each group) / attnB (heads 2,3); the out projection
                # compensates with a row-permuted, zero-padded Wo.
                attnA = singles.tile([128, 2 * L], bf16)  # col = chunk*1024 + l
                attnB = singles.tile([128, 2 * L], bf16)
                nc.vector.memset(attnA[:], 0.0)
                nc.vector.memset(attnB[:], 0.0)
                for g in range(2):
                    for lh in range(2):
                        pts = prefills.pop((g, lh), None)
                        av33a = psp.tile([128, 512], f32, tag="ps")
                        av33b = psp.tile([128, 512], f32, tag="ps")
                        av33 = [av33a, av33b]
                        nc.vector.memset(av33a[:], 0.0)
                        nc.vector.memset(av33b[:], 0.0)
                        for tc_i in range(8):
                            for jp in range(2):  # head pairs (0,1), (2,3)
                                if pts is None:
                                    pt = score_exp(g, lh, tc_i, jp)
                                else:
                                    pt = pts[tc_i * 2 + jp]
                                for jj in range(2):
                                    j = 2 * jp + jj
                                    pos = 64 * (j % 2)
                                    nc.tensor.matmul(
                                        av33[j // 2][pos: pos + 33, :],
                                        v1_sb[:, tc_i, g, j, :], pt[:, jj, :],
                                        start=False, stop=False,
                                        tile_position=(0, pos),
                                        skip_group_check=True)
                        # den rows (32, 96) -> SBUF so the broadcast matmuls
                        # can consume them as moving data
                        dsb = repp.tile([128, 2, 512], bf16, tag="dsb")
                        for ti in range(2):
                            for pb in (32, 96):
                                nc.vector.tensor_copy(
                                    out=dsb[pb: pb + 1, ti, :],
                                    in_=av33[ti][pb: pb + 1, :])
                        gcol = slice(g * L + lh * 512, g * L + (lh + 1) * 512)
                        for ti, attnX in ((0, attnA), (1, attnB)):
                            repb = psp.tile([128, 512], f32, tag="ps")
                            # PE tiling is strictly 1-D (row- OR col-banded),
                            # so the odd head's den (row 96 -> rows 64-95)
                            # broadcasts in hops of valid shapes: (96,0) M=128
                            # floods all rows, a full copy stages to SBUF,
                            # then a (0,64) col-banded matmul with a
                            # 1/128-summing stationary rebuilds rows 64-95;
                            # (32,0) finally overwrites rows 0-31 with the
                            # even head's den.
                            nc.tensor.matmul(
                                repb[:, :], onesb[96:97, :],
                                dsb[96:97, ti, :],
                                start=True, stop=True,
                                tile_position=(96, 0), skip_group_check=True)
                            s32 = repp.tile([128, 512], bf16, tag="rsb")
                            nc.vector.tensor_copy(out=s32[:], in_=repb[:])
                            nc.tensor.matmul(
                                repb[64:128, :], sc128b[:, :], s32[:],
                                start=True, stop=True,
                                tile_position=(0, 64), skip_group_check=True)
                            nc.tensor.matmul(
                                repb[0:32, :], onesb[32:33, 0:32],
                                dsb[32:33, ti, :],
                                start=True, stop=True,
                                tile_position=(32, 0), skip_group_check=True)
                            rsb = repp.tile([128, 512], f32, tag="rsb")
                            # custom-DVE ops misbehave at nonzero base
                            # partition; run the reciprocal over the full
                            # tile (rows 32-63/96-127 hold finite den-scale
                            # junk, never read)
                            nc.vector.reciprocal_approx_fast(
                                out=rsb[:], in_=repb[:])
                            if DEBUG and g == 0 and lh == 0 and ti == 0:
                                dsc = repp.tile([128, 512], f32, tag="rsb")
                                nc.vector.tensor_copy(out=dsc[:], in_=av33[0][:])
                                nc.sync.dma_start(out=dbg["dav"][:], in_=dsc[:])
                                nc.sync.dma_start(out=dbg["ddsb"][:], in_=dsb[:])
                                dsc2 = repp.tile([128, 512], f32, tag="rsb")
                                nc.vector.tensor_copy(out=dsc2[:], in_=repb[:])
                                nc.sync.dma_start(out=dbg["drepb"][:], in_=dsc2[:])
                            for hb in (0, 64):
                                nc.vector.tensor_mul(
                                    attnX[hb: hb + 32, gcol],
                                    av33[ti][hb: hb + 32, :],
                                    rsb[hb: hb + 32, :])

                if DEBUG:
                    nc.sync.dma_start(out=dbg["daoT"][:], in_=attnA[:])
                    nc.sync.dma_start(out=dbg["drsb"][:], in_=rsb[:])

                # ---------- output projection (transposed) + bias ----------
                # po_sb rows are permuted/zero-padded to match the attnA/B
                # row layout, so rows 32-63/96-127 (zeroed at memset) multiply
                # by zero weights and the contraction can stay K=128.
                out_sb = singles.tile([128, 2 * L], f32)
                for lh in range(2):
                    for co in range(2):
                        ps = psp.tile([128, 512], f32, tag="ps")
                        idx = 0
                        for tX, attnX in ((0, attnA), (1, attnB)):
                            for gci in range(2):
                                lhsT = po_sb[:, tX, gci, co * 128:(co + 1) * 128]
                                rhs = attnX[:, gci * L + lh * 512:
                                            gci * L + (lh + 1) * 512]
                                nc.tensor.matmul(ps[:], lhsT, rhs,
                                                 start=(idx == 0), stop=(idx == 3))
                                idx += 1
                        nc.scalar.activation(
                            out_sb[:, co * L + lh * 512: co * L + (lh + 1) * 512],
                            ps[:], AF.Identity, bias=bot[:, co: co + 1], scale=1.0)

                outr = out.rearrange("(c p) l -> p c l", p=128)
                osr = out_sb[:].rearrange("p (c l) -> p c l", l=L)
                for lh in range(2):
                    nc.sync.dma_start(out=outr[:, :, lh * 512:(lh + 1) * 512],
                                      in_=osr[:, :, lh * 512:(lh + 1) * 512])

    nc.compile()
    return nc


WSCALE = 16.0


def _f8(a):
    import ml_dtypes
    return np.ascontiguousarray(a).astype(ml_dtypes.float8_e4m3)


def _prep_weights(conv_q_w, conv_k_w, conv_v_w, Wq, Wk, Wv, Wo,
                  bn_q_g, bn_q_b, bn_k_g, bn_k_b, bn_v_g, bn_v_b, bo):
    def conv_tiles(w):
        # [co, ci, ky, kx] -> [9, 2(ci), 2(co), 128, 128]
        t = np.ascontiguousarray(np.transpose(np.asarray(w, np.float32), (2, 3, 1, 0)))
        t = t.reshape(3, 3, 2, 128, 2, 128).transpose(0, 1, 2, 4, 3, 5)
        return np.ascontiguousarray(t.reshape(9, 2, 2, 128, 128))

    def conv_tiles8(w):
        # [co, ci, ky, kx] -> [9, 2(co), 128(ci_in), 2(ci ch), 128(co_in)] fp8
        t = np.transpose(np.asarray(w, np.float32), (2, 3, 1, 0))  # ky kx ci co
        t = t.reshape(3, 3, 2, 128, 2, 128)        # ky kx cic cip coc cof
        t = t.transpose(0, 1, 4, 3, 2, 5)          # ky kx coc cip cic cof
        return _f8(t.reshape(9, 2, 128, 2, 128) * WSCALE)

    def proj_tiles(w):
        return np.ascontiguousarray(
            np.asarray(w, np.float32).T.reshape(2, 128, C))

    def proj_tiles8(w):
        return _f8(np.asarray(w, np.float32).T.reshape(2, 128, C) * WSCALE)

    def po_tiles(w):
        # row-permuted, zero-padded Wo^T matching the attnA/B row layout
        import ml_dtypes
        wt = np.asarray(w, np.float32).T  # [cin, fo]
        p = np.zeros((2, 2, 128, C), np.float32)
        for tX in range(2):
            for g in range(2):
                for pb in (0, 64):
                    j = tX * 2 + pb // 64
                    p[tX, g, pb: pb + 32, :] = wt[g * 128 + j * 32:
                                                  g * 128 + (j + 1) * 32, :]
        return np.ascontiguousarray(p).astype(ml_dtypes.bfloat16)

    gbp = np.zeros((128, 12), np.float32)
    for i, (g, b) in enumerate(((bn_q_g, bn_q_b), (bn_k_g, bn_k_b), (bn_v_g, bn_v_b))):
        g = np.asarray(g, np.float32).reshape(2, 128)
        b = np.asarray(b, np.float32).reshape(2, 128)
        for ch in range(2):
            gbp[:, 2 * i + ch] = g[ch]
            gbp[:, 6 + 2 * i + ch] = b[ch]
    bop = np.ascontiguousarray(np.asarray(bo, np.float32).reshape(2, 128).T)
    return {
        "wcq": conv_tiles8(conv_q_w), "wck": conv_tiles8(conv_k_w),
        "wcv": conv_tiles(conv_v_w),
        "pq": proj_tiles8(Wq), "pk": proj_tiles8(Wk), "pv": proj_tiles(Wv),
        "po": po_tiles(Wo),
        "gb": gbp, "bo": bop,
    }


def _get_nc(repeat=1):
    key = ("nc", repeat, VARIANT, DEBUG)
    if key not in _CACHE:
        _CACHE[key] = _build_nc(repeat)
    return _CACHE[key]


def run_spmd(in_maps, repeat=1, **kw):
    from concourse.bass_utils import run_bass_kernel_spmd
    return run_bass_kernel_spmd(_get_nc(repeat), in_maps, list(range(8)), **kw)


def _get_executor(repeat=1):
    """Build the sharded jitted callable once (mirrors
    bass2jax.run_bass_via_pjrt's multi-core path) so repeated calls skip
    retracing/compilation."""
    key = ("exec", repeat, VARIANT)
    if key in _CACHE:
        return _CACHE[key]
    import jax
    import numpy as _np
    from jax.sharding import Mesh, PartitionSpec
    from jax.experimental.shard_map import shard_map
    from concourse import bass2jax, mybir

    nc = _get_nc(repeat)
    bass2jax.install_neuronx_cc_hook()
    partition_name = nc.partition_id_tensor.name if nc.partition_id_tensor else None

    in_names, out_names, out_avals, zero_outs = [], [], [], []
    for alloc in nc.m.functions[0].allocations:
        if not isinstance(alloc, mybir.MemoryLocationSet):
            continue
        name = alloc.memorylocations[0].name
        if alloc.kind == "ExternalInput":
            if name != partition_name:
                in_names.append(name)
        elif alloc.kind == "ExternalOutput":
            dt_np = mybir.dt.np(alloc.dtype)
            shape = tuple(alloc.tensor_shape)
            out_avals.append(jax.core.ShapedArray(shape, dt_np))
            out_names.append(name)
            zero_outs.append(_np.zeros(shape, dt_np))

    n_params = len(in_names)
    n_outs = len(out_names)
    all_in_names = list(in_names) + list(out_names)
    if partition_name is not None:
        all_in_names.append(partition_name)
    donate = tuple(range(n_params, n_params + n_outs))

    def _body(*args):
        operands = list(args)
        if partition_name is not None:
            operands.append(bass2jax.partition_id_tensor())
        outs = bass2jax._bass_exec_p.bind(
            *operands,
            out_avals=tuple(out_avals),
            in_names=tuple(all_in_names),
            out_names=tuple(out_names),
            lowering_input_output_aliases=(),
            sim_require_finite=True,
            sim_require_nnan=True,
            nc=nc,
        )
        return tuple(outs)

    devices = jax.devices()[:B]
    mesh = Mesh(np.asarray(devices), ("core",))
    in_specs = (PartitionSpec("core"),) * (n_params + n_outs)
    out_specs = (PartitionSpec("core"),) * n_outs
    sharded = jax.jit(
        shard_map(_body, mesh=mesh, in_specs=in_specs, out_specs=out_specs,
                  check_rep=False),
        donate_argnums=donate, keep_unused=True,
    )
    _CACHE[("mesh", repeat, VARIANT)] = mesh
    _CACHE[("jit", repeat, VARIANT)] = sharded

    def run(in_maps):
        concat_in = [
            np.concatenate([np.asarray(in_maps[c][k]) for c in range(B)], axis=0)
            for k in in_names
        ]
        concat_zeros = [np.zeros((B * z.shape[0], *z.shape[1:]), z.dtype)
                        for z in zero_outs]
        out_arrs = sharded(*concat_in, *concat_zeros)
        return out_arrs, out_names, out_avals

    _CACHE[key] = run
    return run


def run_fast(in_maps, repeat=1):
    """Execute via the cached jitted callable; returns per-core dict list."""
    run = _get_executor(repeat)
    out_arrs, out_names, out_avals = run(in_maps)
    return [
        {name: np.asarray(out_arrs[i]).reshape(B, *out_avals[i].shape)[c]
         for i, name in enumerate(out_names)}
        for c in range(B)
    ]


def bench_wall(in_maps, repeat, n_iter):
    """Dispatch n_iter executions of the repeat-R NEFF with device-resident
    inputs and pre-staged donated zero buffers; return total wall seconds.
    Host/RPC overhead is identical across R, so (wall(R2)-wall(R1)) isolates
    device time."""
    import time as _time
    import jax
    from jax.sharding import NamedSharding, PartitionSpec

    _get_executor(repeat)  # ensure built
    nc = _get_nc(repeat)
    from concourse import mybir
    partition_name = nc.partition_id_tensor.name if nc.partition_id_tensor else None
    in_names, out_shapes = [], []
    for alloc in nc.m.functions[0].allocations:
        if not isinstance(alloc, mybir.MemoryLocationSet):
            continue
        name = alloc.memorylocations[0].name
        if alloc.kind == "ExternalInput" and name != partition_name:
            in_names.append(name)
        elif alloc.kind == "ExternalOutput":
            out_shapes.append((tuple(alloc.tensor_shape), mybir.dt.np(alloc.dtype)))

    key = ("bench_in", repeat, VARIANT)
    if key not in _CACHE:
        run = _CACHE[("exec", repeat, VARIANT)]
        # reach into the executor's jitted fn? rebuild inputs here instead
        mesh = _CACHE[("mesh", repeat, VARIANT)]
        sh = NamedSharding(mesh, PartitionSpec("core"))
        dev_in = [
            jax.device_put(
                np.concatenate([np.asarray(in_maps[c][k]) for c in range(B)], 0), sh)
            for k in in_names
        ]
        _CACHE[key] = (dev_in, sh)
    dev_in, sh = _CACHE[key]

    sharded = _CACHE[("jit", repeat, VARIANT)]
    # pre-stage donated zero sets
    zero_sets = []
    for _ in range(n_iter):
        zs = [jax.device_put(np.zeros((B * s[0], *s[1:]), dt), sh)
              for (s, dt) in out_shapes]
        zero_sets.append(zs)
    for zs in zero_sets:
        for z in zs:
            z.block_until_ready()

    outs = []
    t0 = _time.perf_counter()
    for it in range(n_iter):
        outs.append(sharded(*dev_in, *zero_sets[it]))
    for o in outs[-1]:
        o.block_until_ready()
    t1 = _time.perf_counter()
    return t1 - t0


def make_in_maps(x, y, h, w, conv_q_w, bn_q_g, bn_q_b,
                 conv_k_w, bn_k_g, bn_k_b, conv_v_w, bn_v_g, bn_v_b,
                 Wq, Wk, Wv, Wo, bo):
    assert int(h) == IMG and int(w) == IMG
    x = np.asarray(x, np.float32)
    y = np.asarray(y, np.float32)
    wmap = _prep_weights(conv_q_w, conv_k_w, conv_v_w, Wq, Wk, Wv, Wo,
                         bn_q_g, bn_q_b, bn_k_g, bn_k_b, bn_v_g, bn_v_b, bo)
    def pad_t(a):
        # [B, L, C] -> [B, C, 34*34] with zero border baked in
        at = np.transpose(a, (0, 2, 1)).reshape(B, C, IMG, IMG)
        ap = np.zeros((B, C, PAD, PAD), np.float32)
        ap[:, :, 1:33, 1:33] = at
        return ap.reshape(B, C, PAD * PAD)

    xT = _f8(pad_t(x))
    yT = pad_t(y)
    yT8 = _f8(yT)
    return [dict(wmap, xt=xT[b], yt8=yT8[b], ytr=yT[b]) for b in range(B)]


def kernel(**inputs):
    in_maps = make_in_maps(**inputs)
    res = run_fast(in_maps)
    outs = [res[b]["out"] for b in range(B)]  # each [C, L]
    return np.ascontiguousarray(
        np.stack(outs, axis=0).transpose(0, 2, 1)).astype(np.float32)

